# revision 10
# baseline (speedup 1.0000x reference)
"""CoAttention kernel for 8 Trainium2 NeuronCores.

Sharding: data-parallel over batch B=8 -> one batch per core. BatchNorm
batch-stats are computed per-core and summed with a mid-kernel AllReduce
(sum / sum-of-squares per channel, 2 x [128,12] f32 per branch).

v3 schedule:
  - Branch-a-critical inputs (q tiles 0-3, x_a, both W) load as f32 over
    the fast HWDGE scalar ring and are cast to bf16 on DVE; later-needed
    tensors (q tiles 4-15, x_v) go through the slower SWDGE cast ring in
    parallel.  All xbar transposes stay on the sync ring (shared-xbar
    corruption when two rings transpose concurrently), ordered to match
    branch_a's consumption.
  - Emission: branch_a -> AR_a -> branch_v -> AR_v -> coefs_a -> apply_a
    -> coefs_v -> (apply_v(lb) interleaved with LN-tail(lb)).  AR_a hides
    under branch_v, AR_v under apply_a's relus.
  - apply_a folds the x residual in as x^T (the qT tiles), so the tail
    needs no natural-layout x re-read.  acc is kept per l-block so tail
    deps are block-granular.  LN per l-tile: 6 PE transposes into ONE
    PSUM bank (start-flag zero-region trick), bn_stats on the PSUM tile,
    one fused ACT normalize (scale=rstd, bias=-mu*rstd), gamma on GpSimd,
    beta on DVE, out over sync ring.
"""
import os
import sys

for _p in ("/opt/trn_rl_repo",):
    if _p not in sys.path and os.path.isdir(_p):
        sys.path.append(_p)

import numpy as np

import concourse.bass as bass
import concourse.mybir as mybir
import concourse.tile as tile
from concourse import bacc
from concourse.bass_utils import run_bass_kernel_spmd
from concourse.masks import make_identity

L, B, D = 2048, 8, 768
N_CORES = 8
LT = L // 128          # 16 l-tiles (128 queries)
DT = D // 128          # 6 d-tiles
MT = L // 128          # 16 m-tiles (keys)
LBS = 512              # l-block size
NLB = L // LBS         # 4 l-blocks
CH = 4                 # l-tiles per bf16 nat chunk
NCH = LT // CH
SCH = 2                # l-tiles per f32 staging chunk
WCH = 2                # r-tiles per W chunk
EPS_BN = 1e-5
EPS_LN = 1e-5
SCALE = 1.0 / float(np.sqrt(D))
F32 = mybir.dt.float32
BF16 = mybir.dt.bfloat16
AF = mybir.ActivationFunctionType
ALU = mybir.AluOpType

_CACHED_NC = None


def _build_nc():
    nc = bacc.Bacc("TRN2", target_bir_lowering=False, debug=False,
                   num_devices=N_CORES)

    xq_d = nc.dram_tensor("xq", [L, D], F32, kind="ExternalInput")
    xa_d = nc.dram_tensor("xa", [L, D], F32, kind="ExternalInput")
    xv_d = nc.dram_tensor("xv", [L, D], F32, kind="ExternalInput")
    Wa_d = nc.dram_tensor("Wa", [D, D], F32, kind="ExternalInput")
    Wv_d = nc.dram_tensor("Wv", [D, D], F32, kind="ExternalInput")
    ba_d = nc.dram_tensor("ba", [D], F32, kind="ExternalInput")
    bv_d = nc.dram_tensor("bv", [D], F32, kind="ExternalInput")
    bnag_d = nc.dram_tensor("bnag", [D], F32, kind="ExternalInput")
    bnab_d = nc.dram_tensor("bnab", [D], F32, kind="ExternalInput")
    bnvg_d = nc.dram_tensor("bnvg", [D], F32, kind="ExternalInput")
    bnvb_d = nc.dram_tensor("bnvb", [D], F32, kind="ExternalInput")
    pa_d = nc.dram_tensor("pa", [1], F32, kind="ExternalInput")
    pv_d = nc.dram_tensor("pv", [1], F32, kind="ExternalInput")
    lng_d = nc.dram_tensor("lng", [D], F32, kind="ExternalInput")
    lnb_d = nc.dram_tensor("lnb", [D], F32, kind="ExternalInput")
    out_d = nc.dram_tensor("out", [L, D], F32, kind="ExternalOutput")

    def bcast_ap(t, n):
        a = t.ap() if hasattr(t, "ap") and callable(getattr(t, "ap")) else t
        return bass.AP(tensor=a.tensor, offset=a.offset,
                       ap=[[0, 128]] + [list(x) for x in a.ap])

    from contextlib import ExitStack
    with ExitStack() as ctx:
        tc = ctx.enter_context(tile.TileContext(nc))
        constp = ctx.enter_context(tc.tile_pool(name="const", bufs=1))
        stg32p = ctx.enter_context(tc.tile_pool(name="stg32", bufs=2))   # [128,2,768] f32
        natqp = ctx.enter_context(tc.tile_pool(name="natq", bufs=2))     # [128,4,768] bf16
        natap = ctx.enter_context(tc.tile_pool(name="nata", bufs=4))     # [128,4,768] bf16 (+acc)
        natvp = ctx.enter_context(tc.tile_pool(name="natv", bufs=4))
        wbfp = ctx.enter_context(tc.tile_pool(name="wbf", bufs=3))       # [128,2,768] bf16
        qtp = ctx.enter_context(tc.tile_pool(name="qt", bufs=4))         # [128,6,512] bf16
        kvtp = ctx.enter_context(tc.tile_pool(name="kvt", bufs=16))      # [128,6,128] bf16
        wtp = ctx.enter_context(tc.tile_pool(name="wt", bufs=2))         # [128,6,768] bf16
        ep = ctx.enter_context(tc.tile_pool(name="e", bufs=1))           # [128,16,512] bf16
        ctxp = ctx.enter_context(tc.tile_pool(name="ctx", bufs=6))       # [128,512] bf16
        rbp = ctx.enter_context(tc.tile_pool(name="rb", bufs=1))         # [128,512] f32
        ysp = ctx.enter_context(tc.tile_pool(name="ys", bufs=2))         # [128,6,512] bf16
        ybkp = ctx.enter_context(tc.tile_pool(name="ybk", bufs=1))       # [128,6,512] bf16
        rtp = ctx.enter_context(tc.tile_pool(name="rtmp", bufs=2))       # [128,512] bf16
        statp = ctx.enter_context(tc.tile_pool(name="stats", bufs=1))
        smallp = ctx.enter_context(tc.tile_pool(name="small", bufs=1))
        onatp = ctx.enter_context(tc.tile_pool(name="onat", bufs=2))     # [128,768] f32
        lnsp = ctx.enter_context(tc.tile_pool(name="lns", bufs=3))
        dramp = ctx.enter_context(tc.tile_pool(name="dram", bufs=1, space="DRAM"))
        ps_s = ctx.enter_context(tc.tile_pool(name="ps_s", bufs=2, space="PSUM"))
        ps_sum = ctx.enter_context(tc.tile_pool(name="ps_sum", bufs=1, space="PSUM"))
        ps_acc = ctx.enter_context(tc.tile_pool(name="ps_acc", bufs=4, space="PSUM"))
        if True:
            # ---------- constants / params ----------
            ident = constp.tile([128, 128], BF16)
            make_identity(nc, ident)
            ones = constp.tile([128, 1], BF16)
            nc.vector.memset(ones[:], 1.0)
            epsbn = constp.tile([128, 1], F32)
            nc.vector.memset(epsbn[:], EPS_BN)
            epsln = constp.tile([128, 1], F32)
            nc.vector.memset(epsln[:], EPS_LN)

            def load_pt(t):  # [D] -> [128, DT] with col et, row p = t[et*128+p]
                s = constp.tile([128, DT], F32)
                nc.scalar.dma_start(out=s[:], in_=t.ap().rearrange("(t p) -> p t", p=128))
                return s

            ba_s, bv_s = load_pt(ba_d), load_pt(bv_d)
            bnag_s, bnab_s = load_pt(bnag_d), load_pt(bnab_d)
            bnvg_s, bnvb_s = load_pt(bnvg_d), load_pt(bnvb_d)
            pa_s = constp.tile([128, 1], F32)
            nc.gpsimd.dma_start(out=pa_s[:], in_=bcast_ap(pa_d, 1))
            pv_s = constp.tile([128, 1], F32)
            nc.gpsimd.dma_start(out=pv_s[:], in_=bcast_ap(pv_d, 1))

            # DRAM bounce buffers
            yD0 = dramp.tile([128, DT, L], BF16, tag="yD0")
            yD1 = dramp.tile([128, DT, L], BF16, tag="yD1")
            arin0 = dramp.tile([128, 2 * DT], F32, tag="arin0")
            arin1 = dramp.tile([128, 2 * DT], F32, tag="arin1")
            arout0 = dramp.tile([128, 2 * DT], F32, tag="arout0")
            arout1 = dramp.tile([128, 2 * DT], F32, tag="arout1")
            yD = {0: yD0, 1: yD1}
            arin = {0: arin0, 1: arin1}
            arout = {0: arout0, 1: arout1}

            # ---------- fast-path loads: scalar HWDGE f32 + DVE cast ----------
            # natq chunk0, nat_a (all), Wa, Wv.
            natq = [None] * NCH
            nata = [None] * NCH
            natv = [None] * NCH
            wbf = {}

            natq[0] = natqp.tile([128, CH, D], BF16, tag="natq", name="natq0")
            for c in range(NCH):
                nata[c] = natap.tile([128, CH, D], BF16, tag="nata", name=f"nata{c}")
            for bi in (0, 1):
                for c in range(3):
                    wbf[(bi, c)] = wbfp.tile([128, WCH, D], BF16, tag="wbf",
                                             name=f"wbf{bi}_{c}")

            def hw_load_cast(dst_slice, src_d, row0, nt, name):
                stg = stg32p.tile([128, SCH, D], F32, tag="stg32", name=name)
                src = src_d.ap()[row0:row0 + nt * 128, :]
                nc.scalar.dma_start(out=stg[:, 0:nt, :],
                                    in_=src.rearrange("(t p) d -> p t d", p=128))
                nc.vector.tensor_copy(dst_slice, stg[:, 0:nt, :])

            hw_load_cast(natq[0][:, 0:2, :], xq_d, 0, 2, "sq0a")
            hw_load_cast(natq[0][:, 2:4, :], xq_d, 2 * 128, 2, "sq0b")
            for c in range(NCH):
                hw_load_cast(nata[c][:, 0:2, :], xa_d, (c * CH) * 128, 2, f"sa{c}a")
                hw_load_cast(nata[c][:, 2:4, :], xa_d, (c * CH + 2) * 128, 2, f"sa{c}b")
            for bi, W_d in ((0, Wa_d), (1, Wv_d)):
                for c in range(3):
                    hw_load_cast(wbf[(bi, c)][:], W_d, c * WCH * 128, 2, f"sw{bi}{c}")

            # ---------- slow-path loads: gpsimd SWDGE cast ----------
            def sw_cast_chunk(pool, src_d, c, nt, name):
                t = pool.tile([128, nt, D], BF16, tag=pool.name, name=name)
                src = src_d.ap()[c * nt * 128:(c + 1) * nt * 128, :]
                nc.gpsimd.dma_start(
                    out=t[:], in_=src.rearrange("(t p) d -> p t d", p=128))
                return t

            for c in range(1, NCH):
                natq[c] = sw_cast_chunk(natqp, xq_d, c, CH, f"natq{c}")
            for c in range(NCH):
                natv[c] = sw_cast_chunk(natvp, xv_d, c, CH, f"natv{c}")
            lng_s = constp.tile([128, D], BF16)
            nc.gpsimd.dma_start(out=lng_s[:], in_=bcast_ap(lng_d, D))
            lnb_s = constp.tile([128, D], BF16)
            nc.gpsimd.dma_start(out=lnb_s[:], in_=bcast_ap(lnb_d, D))

            def nat_slice(chunks, g):          # [128, 768] natural l-tile g
                return chunks[g // CH][:, g % CH, :]

            # ---------- transposes (ALL on the sync HWDGE ring) ----------
            kvT_a = [kvtp.tile([128, DT, 128], BF16, tag="kvT", name=f"kvTa{c}")
                     for c in range(MT)]
            kvT_v = [kvtp.tile([128, DT, 128], BF16, tag="kvT", name=f"kvTv{c}")
                     for c in range(MT)]
            qT = [qtp.tile([128, DT, LBS], BF16, tag="qT", name=f"qT{i}")
                  for i in range(NLB)]

            def tq(c):
                nc.sync.dma_start_transpose(
                    qT[c // CH][:, :, (c % CH) * 128:(c % CH + 1) * 128],
                    nat_slice(natq, c))

            def wt_transpose(bi):
                WT = wtp.tile([128, DT, D], BF16, tag="WT", name=f"WT{bi}")
                for c6 in range(DT):
                    nc.sync.dma_start_transpose(
                        WT[:, :, c6 * 128:(c6 + 1) * 128],
                        wbf[(bi, c6 // WCH)][:, c6 % WCH, :])
                return WT

            for c in range(CH):
                tq(c)
            for c in range(MT):
                nc.sync.dma_start_transpose(kvT_a[c][:, :, :], nat_slice(nata, c))
            WTa = wt_transpose(0)
            WTv = wt_transpose(1)
            for c in range(CH, LT):
                tq(c)
            for c in range(MT):
                nc.sync.dma_start_transpose(kvT_v[c][:, :, :], nat_slice(natv, c))

            # ---------- branch compute (S -> softmax -> ctx -> y -> stats) ----
            def branch_compute(bi, WT, b_s, kvT, nat_kv):
                statsr = statp.tile([128, DT, NLB, 6], F32, tag=f"statsr{bi}")
                for lb in range(NLB):
                    E = ep.tile([128, MT, LBS], BF16, tag="E")
                    for mt in range(MT):
                        S = ps_s.tile([128, LBS], F32, tag="S")
                        for dt in range(DT):
                            nc.tensor.matmul(
                                S[:], kvT[mt][:, dt, :], qT[lb][:, dt, :],
                                start=(dt == 0), stop=(dt == DT - 1))
                        nc.scalar.activation(out=E[:, mt, :], in_=S[:],
                                             func=AF.Exp, scale=SCALE)
                    s_ps = ps_sum.tile([1, LBS], F32, tag="ssum")
                    for mt in range(MT):
                        nc.tensor.matmul(s_ps[:], ones[:], E[:, mt, :],
                                         start=(mt == 0), stop=(mt == MT - 1))
                    rsb = smallp.tile([1, LBS], F32, tag="rsb")
                    nc.vector.reciprocal(rsb[:], s_ps[:])
                    rbd = dramp.tile([LBS], F32, tag="rbd")
                    nc.scalar.dma_start(out=rbd[:], in_=rsb[:])
                    rbc = rbp.tile([128, LBS], F32, tag="rbc")
                    nc.scalar.dma_start(out=rbc[:], in_=bcast_ap(rbd, LBS))

                    ctx_sb = []
                    for dt in range(DT):
                        cps = ps_acc.tile([128, LBS], F32, tag="acc")
                        for mt in range(MT):
                            nc.tensor.matmul(
                                cps[:], nat_slice(nat_kv, mt)[:, dt * 128:(dt + 1) * 128],
                                E[:, mt, :], start=(mt == 0), stop=(mt == MT - 1))
                        csb = ctxp.tile([128, LBS], BF16, tag="ctx")
                        nc.vector.tensor_copy(csb[:], cps[:])
                        ctx_sb.append(csb)
                    ysb = ysp.tile([128, DT, LBS], BF16, tag="ys")
                    for et in range(DT):
                        yps = ps_acc.tile([128, LBS], F32, tag="acc")
                        for dt in range(DT):
                            nc.tensor.matmul(
                                yps[:], WT[:, dt, et * 128:(et + 1) * 128],
                                ctx_sb[dt][:], start=(dt == 0), stop=(dt == DT - 1))
                        nc.vector.tensor_mul(ysb[:, et, :], yps[:], rbc[:])
                        nc.vector.tensor_scalar(
                            out=ysb[:, et, :], in0=ysb[:, et, :],
                            scalar1=b_s[:, et:et + 1], scalar2=None, op0=ALU.add)
                        nc.vector.bn_stats(out=statsr[:, et, lb, :], in_=ysb[:, et, :])
                    nc.gpsimd.dma_start(
                        out=yD[bi][:, :, lb * LBS:(lb + 1) * LBS], in_=ysb[:])

                # per-core stats -> sums -> AllReduce kickoff
                mv = smallp.tile([128, DT, 2], F32, tag=f"mv{bi}")
                for et in range(DT):
                    nc.vector.bn_aggr(out=mv[:, et, :], in_=statsr[:, et, :, :])
                arin_s = smallp.tile([128, 2 * DT], F32, tag=f"ari{bi}")
                nc.vector.tensor_scalar(
                    out=arin_s[:, 0:DT], in0=mv[:, :, 0], scalar1=float(L),
                    scalar2=None, op0=ALU.mult)
                tmp = smallp.tile([128, DT], F32, tag=f"tmp{bi}")
                nc.vector.tensor_mul(tmp[:], mv[:, :, 0], mv[:, :, 0])
                nc.vector.tensor_add(tmp[:], tmp[:], mv[:, :, 1])
                nc.vector.tensor_scalar(
                    out=arin_s[:, DT:2 * DT], in0=tmp[:], scalar1=float(L),
                    scalar2=None, op0=ALU.mult)
                nc.scalar.dma_start(out=arin[bi][:], in_=arin_s[:])
                nc.gpsimd.collective_compute(
                    "AllReduce", ALU.add,
                    replica_groups=[list(range(N_CORES))],
                    ins=[arin[bi].opt()], outs=[arout[bi].opt()])

            # ---------- BN coefficients from the AllReduced stats ----------
            def branch_coefs(bi, bng_s, bnb_s, alpha_s):
                gs = smallp.tile([128, 2 * DT], F32, tag=f"gs{bi}")
                nc.scalar.dma_start(out=gs[:], in_=arout[bi][:])
                inv_n = 1.0 / float(L * N_CORES)
                gm = smallp.tile([128, DT], F32, tag=f"gm{bi}")
                nc.vector.tensor_scalar(out=gm[:], in0=gs[:, 0:DT],
                                        scalar1=inv_n, scalar2=None, op0=ALU.mult)
                gvar = smallp.tile([128, DT], F32, tag=f"gv{bi}")
                nc.vector.tensor_scalar(out=gvar[:], in0=gs[:, DT:2 * DT],
                                        scalar1=inv_n, scalar2=None, op0=ALU.mult)
                tmp2 = smallp.tile([128, DT], F32, tag=f"t2{bi}")
                nc.vector.tensor_mul(tmp2[:], gm[:], gm[:])
                nc.vector.tensor_sub(gvar[:], gvar[:], tmp2[:])
                std = smallp.tile([128, DT], F32, tag=f"sd{bi}")
                nc.scalar.activation(out=std[:], in_=gvar[:], func=AF.Sqrt,
                                     bias=epsbn[:], scale=1.0)
                rstd = smallp.tile([128, DT], F32, tag=f"rs{bi}")
                nc.vector.reciprocal(rstd[:], std[:])
                sc1 = smallp.tile([128, DT], F32, tag=f"s1{bi}")
                nc.vector.tensor_mul(sc1[:], bng_s[:], rstd[:])
                sh1 = smallp.tile([128, DT], F32, tag=f"h1{bi}")
                nc.vector.tensor_mul(sh1[:], gm[:], sc1[:])
                nc.vector.tensor_sub(sh1[:], bnb_s[:], sh1[:])
                sc2 = smallp.tile([128, DT], F32, tag=f"s2{bi}")
                nc.vector.tensor_scalar(out=sc2[:], in0=sc1[:], scalar1=alpha_s[:],
                                        scalar2=-1.0, op0=ALU.mult, op1=ALU.mult)
                sh2 = smallp.tile([128, DT], F32, tag=f"h2{bi}")
                nc.vector.tensor_scalar(out=sh2[:], in0=sh1[:], scalar1=alpha_s[:],
                                        scalar2=-1.0, op0=ALU.mult, op1=ALU.mult)
                return sc1, sh1, sc2, sh2

            # ---------- BN + PReLU apply (+x^T fold for branch a) ----------
            acc_lb = [None] * NLB

            def apply_lb(bi, coefs, lc):
                sc1, sh1, sc2, sh2 = coefs
                lsl = slice(lc * LBS, (lc + 1) * LBS)
                if bi == 0:
                    acc_lb[lc] = natap.tile([128, DT, LBS], BF16, tag="nata",
                                            name=f"acc{lc}")
                acc = acc_lb[lc]
                ybk = ybkp.tile([128, DT, LBS], BF16, tag="ybk")
                nc.scalar.dma_start(out=ybk[:], in_=yD[bi][:, :, lsl])
                for et in range(DT):
                    r1 = rtp.tile([128, LBS], BF16, tag="rt")
                    nc.scalar.activation(out=r1[:], in_=ybk[:, et, :], func=AF.Relu,
                                         scale=sc1[:, et:et + 1], bias=sh1[:, et:et + 1])
                    r2 = rtp.tile([128, LBS], BF16, tag="rt")
                    nc.scalar.activation(out=r2[:], in_=ybk[:, et, :], func=AF.Relu,
                                         scale=sc2[:, et:et + 1], bias=sh2[:, et:et + 1])
                    if bi == 0:
                        nc.vector.tensor_sub(acc[:, et, :], r1[:], r2[:])
                        nc.vector.tensor_add(acc[:, et, :], acc[:, et, :],
                                             qT[lc][:, et, :])
                    else:
                        nc.vector.tensor_add(acc[:, et, :], acc[:, et, :], r1[:])
                        nc.vector.tensor_sub(acc[:, et, :], acc[:, et, :], r2[:])

            # ---------- LN tail for the 4 l-tiles of one l-block ----------
            def tail_lb(lc):
                for j in range(CH):
                    lt = lc * CH + j
                    tp = ps_acc.tile([128, D], BF16, tag="acc", name=f"tp{lt}")
                    for dt in range(DT):
                        nc.tensor.matmul(
                            tp[:, dt * 128:(dt + 1) * 128],
                            acc_lb[lc][:, dt, j * 128:(j + 1) * 128], ident[:],
                            is_transpose=True, start=(dt == 0), stop=(dt == DT - 1),
                            skip_group_check=True)
                    lns = lnsp.tile([128, 2, 6], F32, tag="lns")
                    for g2 in range(2):
                        nc.vector.bn_stats(out=lns[:, g2, :],
                                           in_=tp[:, g2 * 384:(g2 + 1) * 384])
                    mvl = lnsp.tile([128, 2], F32, tag="mvl")
                    nc.vector.bn_aggr(out=mvl[:], in_=lns[:])
                    stdl = lnsp.tile([128, 1], F32, tag="stdl")
                    nc.scalar.activation(out=stdl[:], in_=mvl[:, 1:2], func=AF.Sqrt,
                                         bias=epsln[:], scale=1.0)
                    rstdl = lnsp.tile([128, 1], F32, tag="rstdl")
                    nc.vector.reciprocal(rstdl[:], stdl[:])
                    nbl = lnsp.tile([128, 1], F32, tag="nbl")
                    nc.vector.tensor_scalar(out=nbl[:], in0=mvl[:, 0:1],
                                            scalar1=rstdl[:], scalar2=-1.0,
                                            op0=ALU.mult, op1=ALU.mult)
                    onat = onatp.tile([128, D], F32, tag="onat")
                    nc.scalar.activation(out=onat[:], in_=tp[:], func=AF.Identity,
                                         scale=rstdl[:], bias=nbl[:])
                    nc.gpsimd.tensor_mul(onat[:], onat[:], lng_s[:])
                    nc.vector.tensor_add(onat[:], onat[:], lnb_s[:])
                    nc.sync.dma_start(out=out_d.ap()[lt * 128:(lt + 1) * 128, :],
                                      in_=onat[:])

            branch_compute(0, WTa, ba_s, kvT_a, nata)
            branch_compute(1, WTv, bv_s, kvT_v, natv)
            coefs_a = branch_coefs(0, bnag_s, bnab_s, pa_s)
            for lc in range(NLB):
                apply_lb(0, coefs_a, lc)
            coefs_v = branch_coefs(1, bnvg_s, bnvb_s, pv_s)
            for lc in range(NLB):
                apply_lb(1, coefs_v, lc)
                tail_lb(lc)

    nc.compile()
    return nc


def _get_nc():
    global _CACHED_NC
    if _CACHED_NC is None:
        _CACHED_NC = _build_nc()
    return _CACHED_NC


def kernel(**inputs):
    nc = _get_nc()
    x_a = np.asarray(inputs["x_a"], np.float32)
    x_v = np.asarray(inputs["x_v"], np.float32)
    x = np.asarray(inputs["x"], np.float32)
    shared = {
        "Wa": np.ascontiguousarray(inputs["W_a"], np.float32),
        "Wv": np.ascontiguousarray(inputs["W_v"], np.float32),
        "ba": np.ascontiguousarray(inputs["b_a"], np.float32),
        "bv": np.ascontiguousarray(inputs["b_v"], np.float32),
        "bnag": np.ascontiguousarray(inputs["bn_a_g"], np.float32),
        "bnab": np.ascontiguousarray(inputs["bn_a_b"], np.float32),
        "bnvg": np.ascontiguousarray(inputs["bn_v_g"], np.float32),
        "bnvb": np.ascontiguousarray(inputs["bn_v_b"], np.float32),
        "pa": np.ascontiguousarray(inputs["prelu_a"], np.float32),
        "pv": np.ascontiguousarray(inputs["prelu_v"], np.float32),
        "lng": np.ascontiguousarray(inputs["ln_g"], np.float32),
        "lnb": np.ascontiguousarray(inputs["ln_b"], np.float32),
    }
    in_maps = []
    for b in range(N_CORES):
        m = dict(shared)
        m["xq"] = np.ascontiguousarray(x[:, b, :])
        m["xa"] = np.ascontiguousarray(x_a[:, b, :])
        m["xv"] = np.ascontiguousarray(x_v[:, b, :])
        in_maps.append(m)
    trace = bool(int(os.environ.get("COATT_TRACE", "0")))
    res = run_bass_kernel_spmd(nc, in_maps, core_ids=list(range(N_CORES)),
                               trace=trace)
    kernel.last_results = res
    out = np.stack([res.results[b]["out"] for b in range(N_CORES)], axis=1)
    return out.astype(np.float32)


# revision 13
# speedup vs baseline: 1.1696x; 1.1696x over previous
"""CoAttention kernel for 8 Trainium2 NeuronCores.

Sharding: data-parallel over batch B=8 -> one batch per core. BatchNorm
batch-stats are computed per-core and summed with a mid-kernel AllReduce
(sum / sum-of-squares per channel, 2 x [128,12] f32 per branch).

v3 schedule:
  - Branch-a-critical inputs (q tiles 0-3, x_a, both W) load as f32 over
    the fast HWDGE scalar ring and are cast to bf16 on DVE; later-needed
    tensors (q tiles 4-15, x_v) go through the slower SWDGE cast ring in
    parallel.  All xbar transposes stay on the sync ring (shared-xbar
    corruption when two rings transpose concurrently), ordered to match
    branch_a's consumption.
  - Emission: branch_a -> AR_a -> branch_v -> AR_v -> coefs_a -> apply_a
    -> coefs_v -> (apply_v(lb) interleaved with LN-tail(lb)).  AR_a hides
    under branch_v, AR_v under apply_a's relus.
  - apply_a folds the x residual in as x^T (the qT tiles), so the tail
    needs no natural-layout x re-read.  acc is kept per l-block so tail
    deps are block-granular.  LN per l-tile: 6 PE transposes into ONE
    PSUM bank (start-flag zero-region trick), bn_stats on the PSUM tile,
    one fused ACT normalize (scale=rstd, bias=-mu*rstd), gamma on GpSimd,
    beta on DVE, out over sync ring.
"""
import os
import sys

for _p in ("/opt/trn_rl_repo",):
    if _p not in sys.path and os.path.isdir(_p):
        sys.path.append(_p)

import numpy as np

import concourse.bass as bass
import concourse.mybir as mybir
import concourse.tile as tile
from concourse import bacc
from concourse.bass_utils import run_bass_kernel_spmd
from concourse.masks import make_identity

L, B, D = 2048, 8, 768
N_CORES = 8
LT = L // 128          # 16 l-tiles (128 queries)
DT = D // 128          # 6 d-tiles
MT = L // 128          # 16 m-tiles (keys)
LBS = 512              # l-block size
NLB = L // LBS         # 4 l-blocks
CH = 4                 # l-tiles per bf16 nat chunk
NCH = LT // CH
SCH = 2                # l-tiles per f32 staging chunk
WCH = 2                # r-tiles per W chunk
EPS_BN = 1e-5
EPS_LN = 1e-5
SCALE = 1.0 / float(np.sqrt(D))
F32 = mybir.dt.float32
BF16 = mybir.dt.bfloat16
AF = mybir.ActivationFunctionType
ALU = mybir.AluOpType

_CACHED_NC = None


def _build_nc():
    nc = bacc.Bacc("TRN2", target_bir_lowering=False, debug=False,
                   num_devices=N_CORES)

    xq_d = nc.dram_tensor("xq", [L, D], F32, kind="ExternalInput")
    xa_d = nc.dram_tensor("xa", [L, D], F32, kind="ExternalInput")
    xv_d = nc.dram_tensor("xv", [L, D], F32, kind="ExternalInput")
    Wa_d = nc.dram_tensor("Wa", [D, D], F32, kind="ExternalInput")
    Wv_d = nc.dram_tensor("Wv", [D, D], F32, kind="ExternalInput")
    ba_d = nc.dram_tensor("ba", [D], F32, kind="ExternalInput")
    bv_d = nc.dram_tensor("bv", [D], F32, kind="ExternalInput")
    bnag_d = nc.dram_tensor("bnag", [D], F32, kind="ExternalInput")
    bnab_d = nc.dram_tensor("bnab", [D], F32, kind="ExternalInput")
    bnvg_d = nc.dram_tensor("bnvg", [D], F32, kind="ExternalInput")
    bnvb_d = nc.dram_tensor("bnvb", [D], F32, kind="ExternalInput")
    pa_d = nc.dram_tensor("pa", [1], F32, kind="ExternalInput")
    pv_d = nc.dram_tensor("pv", [1], F32, kind="ExternalInput")
    lng_d = nc.dram_tensor("lng", [D], F32, kind="ExternalInput")
    lnb_d = nc.dram_tensor("lnb", [D], F32, kind="ExternalInput")
    out_d = nc.dram_tensor("out", [L, D], F32, kind="ExternalOutput")

    def bcast_ap(t, n):
        a = t.ap() if hasattr(t, "ap") and callable(getattr(t, "ap")) else t
        return bass.AP(tensor=a.tensor, offset=a.offset,
                       ap=[[0, 128]] + [list(x) for x in a.ap])

    from contextlib import ExitStack
    with ExitStack() as ctx:
        tc = ctx.enter_context(tile.TileContext(nc))
        constp = ctx.enter_context(tc.tile_pool(name="const", bufs=1))
        stg32p = ctx.enter_context(tc.tile_pool(name="stg32", bufs=2))   # [128,2,768] f32
        natqp = ctx.enter_context(tc.tile_pool(name="natq", bufs=2))     # [128,4,768] bf16
        natap = ctx.enter_context(tc.tile_pool(name="nata", bufs=4))     # [128,4,768] bf16 (+acc)
        natvp = ctx.enter_context(tc.tile_pool(name="natv", bufs=4))
        wbfp = ctx.enter_context(tc.tile_pool(name="wbf", bufs=3))       # [128,2,768] bf16
        qtp = ctx.enter_context(tc.tile_pool(name="qt", bufs=4))         # [128,6,512] bf16
        kvtp = ctx.enter_context(tc.tile_pool(name="kvt", bufs=16))      # [128,6,128] bf16
        wtp = ctx.enter_context(tc.tile_pool(name="wt", bufs=2))         # [128,6,768] bf16
        ep = ctx.enter_context(tc.tile_pool(name="e", bufs=1))           # [128,16,512] bf16
        ctxp = ctx.enter_context(tc.tile_pool(name="ctx", bufs=6))       # [128,512] bf16
        rbp = ctx.enter_context(tc.tile_pool(name="rb", bufs=1))         # [128,512] f32
        ysp = ctx.enter_context(tc.tile_pool(name="ys", bufs=2))         # [128,6,512] bf16
        ybkp = ctx.enter_context(tc.tile_pool(name="ybk", bufs=1))       # [128,6,512] bf16
        rtp = ctx.enter_context(tc.tile_pool(name="rtmp", bufs=4))       # [128,512] bf16
        statp = ctx.enter_context(tc.tile_pool(name="stats", bufs=1))
        smallp = ctx.enter_context(tc.tile_pool(name="small", bufs=1))
        onatp = ctx.enter_context(tc.tile_pool(name="onat", bufs=2))     # [128,768] f32
        lnsp = ctx.enter_context(tc.tile_pool(name="lns", bufs=3))
        dramp = ctx.enter_context(tc.tile_pool(name="dram", bufs=1, space="DRAM"))
        ps_s = ctx.enter_context(tc.tile_pool(name="ps_s", bufs=2, space="PSUM"))
        ps_sum = ctx.enter_context(tc.tile_pool(name="ps_sum", bufs=1, space="PSUM"))
        ps_acc = ctx.enter_context(tc.tile_pool(name="ps_acc", bufs=4, space="PSUM"))
        if True:
            # ---------- constants / params ----------
            ident = constp.tile([128, 128], BF16)
            make_identity(nc, ident)
            ones = constp.tile([128, 1], BF16)
            nc.vector.memset(ones[:], 1.0)
            epsbn = constp.tile([128, 1], F32)
            nc.vector.memset(epsbn[:], EPS_BN)
            epsln = constp.tile([128, 1], F32)
            nc.vector.memset(epsln[:], EPS_LN)

            def load_pt(t, nm):  # [D] -> [128, DT] with col et, row p = t[et*128+p]
                s = constp.tile([128, DT], F32, tag=f"pt_{nm}", name=f"pt_{nm}")
                nc.scalar.dma_start(out=s[:], in_=t.ap().rearrange("(t p) -> p t", p=128))
                return s

            ba_s, bv_s = load_pt(ba_d, "ba"), load_pt(bv_d, "bv")
            bnag_s, bnab_s = load_pt(bnag_d, "bnag"), load_pt(bnab_d, "bnab")
            bnvg_s, bnvb_s = load_pt(bnvg_d, "bnvg"), load_pt(bnvb_d, "bnvb")
            pa_s = constp.tile([128, 1], F32)
            nc.gpsimd.dma_start(out=pa_s[:], in_=bcast_ap(pa_d, 1))
            pv_s = constp.tile([128, 1], F32)
            nc.gpsimd.dma_start(out=pv_s[:], in_=bcast_ap(pv_d, 1))

            # DRAM bounce buffers
            yD0 = dramp.tile([128, DT, L], BF16, tag="yD0")
            yD1 = dramp.tile([128, DT, L], BF16, tag="yD1")
            arin0 = dramp.tile([128, 2 * DT], F32, tag="arin0")
            arin1 = dramp.tile([128, 2 * DT], F32, tag="arin1")
            arout0 = dramp.tile([128, 2 * DT], F32, tag="arout0")
            arout1 = dramp.tile([128, 2 * DT], F32, tag="arout1")
            yD = {0: yD0, 1: yD1}
            arin = {0: arin0, 1: arin1}
            arout = {0: arout0, 1: arout1}

            # ---------- loads ----------
            # Fast path (branch-a-critical): f32 over the sync HWDGE ring
            # (interleaved with the transposes, grouped to minimize
            # xbar-mode transitions), cast to bf16 on DVE.  Slow path
            # (needed later): gpsimd SWDGE cast ring.
            natq = [None] * NCH
            nata = [None] * NCH
            natv = [None] * NCH
            wbf = {}

            natq[0] = natqp.tile([128, CH, D], BF16, tag="natq", name="natq0")
            for c in range(NCH):
                nata[c] = natap.tile([128, CH, D], BF16, tag="nata", name=f"nata{c}")
            for c in range(3):
                wbf[(0, c)] = wbfp.tile([128, WCH, D], BF16, tag="wbf",
                                        name=f"wbf0_{c}")

            def hw_load_cast(dst_slice, src_d, row0, nt, name):
                stg = stg32p.tile([128, SCH, D], F32, tag="stg32", name=name)
                src = src_d.ap()[row0:row0 + nt * 128, :]
                nc.sync.dma_start(out=stg[:, 0:nt, :],
                                  in_=src.rearrange("(t p) d -> p t d", p=128))
                nc.vector.tensor_copy(dst_slice, stg[:, 0:nt, :])

            def sw_cast_chunk(pool, src_d, c, nt, name):
                t = pool.tile([128, nt, D], BF16, tag=pool.name, name=name)
                src = src_d.ap()[c * nt * 128:(c + 1) * nt * 128, :]
                nc.gpsimd.dma_start(
                    out=t[:], in_=src.rearrange("(t p) d -> p t d", p=128))
                return t

            def nat_slice(chunks, g):          # [128, 768] natural l-tile g
                return chunks[g // CH][:, g % CH, :]

            kvT_a = [kvtp.tile([128, DT, 128], BF16, tag="kvT", name=f"kvTa{c}")
                     for c in range(MT)]
            kvT_v = [kvtp.tile([128, DT, 128], BF16, tag="kvT", name=f"kvTv{c}")
                     for c in range(MT)]
            qT = [qtp.tile([128, DT, LBS], BF16, tag="qT", name=f"qT{i}")
                  for i in range(NLB)]

            def tq(c):
                nc.sync.dma_start_transpose(
                    qT[c // CH][:, :, (c % CH) * 128:(c % CH + 1) * 128],
                    nat_slice(natq, c))

            def wt_transpose(bi):
                WT = wtp.tile([128, DT, D], BF16, tag="WT", name=f"WT{bi}")
                for c6 in range(DT):
                    nc.sync.dma_start_transpose(
                        WT[:, :, c6 * 128:(c6 + 1) * 128],
                        wbf[(bi, c6 // WCH)][:, c6 % WCH, :])
                return WT

            # sync ring: copy/transpose groups in consumption order.
            hw_load_cast(natq[0][:, 0:2, :], xq_d, 0, 2, "sq0a")
            hw_load_cast(natq[0][:, 2:4, :], xq_d, 2 * 128, 2, "sq0b")
            for c in range(CH):
                tq(c)
            # gpsimd SWDGE queue: slow-path cast loads, in need-order.
            # (yD bounce writes are emitted inside branch_compute and land
            # between these in ring order.)
            for c in range(1, NCH):
                natq[c] = sw_cast_chunk(natqp, xq_d, c, CH, f"natq{c}")
            for c in (0, 1):
                hw_load_cast(nata[c][:, 0:2, :], xa_d, (c * CH) * 128, 2, f"sa{c}a")
                hw_load_cast(nata[c][:, 2:4, :], xa_d, (c * CH + 2) * 128, 2, f"sa{c}b")
            for c in range(8):
                nc.sync.dma_start_transpose(kvT_a[c][:, :, :], nat_slice(nata, c))
            for c in (2, 3):
                hw_load_cast(nata[c][:, 0:2, :], xa_d, (c * CH) * 128, 2, f"sa{c}a")
                hw_load_cast(nata[c][:, 2:4, :], xa_d, (c * CH + 2) * 128, 2, f"sa{c}b")
            for c in range(3):
                hw_load_cast(wbf[(0, c)][:], Wa_d, c * WCH * 128, 2, f"sw0{c}")
            for c in range(8, MT):
                nc.sync.dma_start_transpose(kvT_a[c][:, :, :], nat_slice(nata, c))
            WTa = wt_transpose(0)
            for c in range(CH, LT):
                tq(c)

            # remaining slow-path loads (gpsimd), then their transposes (sync).
            for c in range(NCH):
                natv[c] = sw_cast_chunk(natvp, xv_d, c, CH, f"natv{c}")
            for c in range(3):
                wv = sw_cast_chunk(wbfp, Wv_d, c, WCH, f"wv{c}")
                wbf[(1, c)] = wv
            lng_s = constp.tile([128, D], BF16)
            nc.gpsimd.dma_start(out=lng_s[:], in_=bcast_ap(lng_d, D))
            lnb_s = constp.tile([128, D], BF16)
            nc.gpsimd.dma_start(out=lnb_s[:], in_=bcast_ap(lnb_d, D))

            for c in range(MT):
                nc.sync.dma_start_transpose(kvT_v[c][:, :, :], nat_slice(natv, c))
            WTv = wt_transpose(1)

            # ---------- branch compute (S -> softmax -> ctx -> y -> stats) ----
            def branch_compute(bi, WT, b_s, kvT, nat_kv):
                statsr = statp.tile([128, DT, NLB, 6], F32, tag=f"statsr{bi}")
                for lb in range(NLB):
                    E = ep.tile([128, MT, LBS], BF16, tag="E")
                    for mt in range(MT):
                        S = ps_s.tile([128, LBS], F32, tag="S")
                        for dt in range(DT):
                            nc.tensor.matmul(
                                S[:], kvT[mt][:, dt, :], qT[lb][:, dt, :],
                                start=(dt == 0), stop=(dt == DT - 1))
                        nc.scalar.activation(out=E[:, mt, :], in_=S[:],
                                             func=AF.Exp, scale=SCALE)
                    s_ps = ps_sum.tile([1, LBS], F32, tag="ssum")
                    for mt in range(MT):
                        nc.tensor.matmul(s_ps[:], ones[:], E[:, mt, :],
                                         start=(mt == 0), stop=(mt == MT - 1))
                    rsb = smallp.tile([1, LBS], F32, tag="rsb")
                    nc.vector.reciprocal(rsb[:], s_ps[:])
                    rbd = dramp.tile([LBS], F32, tag="rbd")
                    nc.scalar.dma_start(out=rbd[:], in_=rsb[:])
                    rbc = rbp.tile([128, LBS], F32, tag="rbc")
                    nc.scalar.dma_start(out=rbc[:], in_=bcast_ap(rbd, LBS))

                    ctx_sb = []
                    for dt in range(DT):
                        cps = ps_acc.tile([128, LBS], F32, tag="acc")
                        for mt in range(MT):
                            nc.tensor.matmul(
                                cps[:], nat_slice(nat_kv, mt)[:, dt * 128:(dt + 1) * 128],
                                E[:, mt, :], start=(mt == 0), stop=(mt == MT - 1))
                        csb = ctxp.tile([128, LBS], BF16, tag="ctx")
                        nc.vector.tensor_copy(csb[:], cps[:])
                        ctx_sb.append(csb)
                    ysb = ysp.tile([128, DT, LBS], BF16, tag="ys")
                    for et in range(DT):
                        yps = ps_acc.tile([128, LBS], F32, tag="acc")
                        for dt in range(DT):
                            nc.tensor.matmul(
                                yps[:], WT[:, dt, et * 128:(et + 1) * 128],
                                ctx_sb[dt][:], start=(dt == 0), stop=(dt == DT - 1))
                        nc.vector.tensor_mul(ysb[:, et, :], yps[:], rbc[:])
                        nc.vector.tensor_scalar(
                            out=ysb[:, et, :], in0=ysb[:, et, :],
                            scalar1=b_s[:, et:et + 1], scalar2=None, op0=ALU.add)
                        nc.vector.bn_stats(out=statsr[:, et, lb, :], in_=ysb[:, et, :])
                    nc.gpsimd.dma_start(
                        out=yD[bi][:, :, lb * LBS:(lb + 1) * LBS], in_=ysb[:])

                # per-core stats -> sums -> AllReduce kickoff
                mv = smallp.tile([128, DT, 2], F32, tag=f"mv{bi}")
                for et in range(DT):
                    nc.vector.bn_aggr(out=mv[:, et, :], in_=statsr[:, et, :, :])
                arin_s = smallp.tile([128, 2 * DT], F32, tag=f"ari{bi}")
                nc.vector.tensor_scalar(
                    out=arin_s[:, 0:DT], in0=mv[:, :, 0], scalar1=float(L),
                    scalar2=None, op0=ALU.mult)
                tmp = smallp.tile([128, DT], F32, tag=f"tmp{bi}")
                nc.vector.tensor_mul(tmp[:], mv[:, :, 0], mv[:, :, 0])
                nc.vector.tensor_add(tmp[:], tmp[:], mv[:, :, 1])
                nc.vector.tensor_scalar(
                    out=arin_s[:, DT:2 * DT], in0=tmp[:], scalar1=float(L),
                    scalar2=None, op0=ALU.mult)
                nc.scalar.dma_start(out=arin[bi][:], in_=arin_s[:])
                nc.gpsimd.collective_compute(
                    "AllReduce", ALU.add,
                    replica_groups=[list(range(N_CORES))],
                    ins=[arin[bi].opt()], outs=[arout[bi].opt()])

            # ---------- BN coefficients from the AllReduced stats ----------
            def branch_coefs(bi, bng_s, bnb_s, alpha_s):
                gs = smallp.tile([128, 2 * DT], F32, tag=f"gs{bi}")
                nc.scalar.dma_start(out=gs[:], in_=arout[bi][:])
                inv_n = 1.0 / float(L * N_CORES)
                gm = smallp.tile([128, DT], F32, tag=f"gm{bi}")
                nc.vector.tensor_scalar(out=gm[:], in0=gs[:, 0:DT],
                                        scalar1=inv_n, scalar2=None, op0=ALU.mult)
                gvar = smallp.tile([128, DT], F32, tag=f"gv{bi}")
                nc.vector.tensor_scalar(out=gvar[:], in0=gs[:, DT:2 * DT],
                                        scalar1=inv_n, scalar2=None, op0=ALU.mult)
                tmp2 = smallp.tile([128, DT], F32, tag=f"t2{bi}")
                nc.vector.tensor_mul(tmp2[:], gm[:], gm[:])
                nc.vector.tensor_sub(gvar[:], gvar[:], tmp2[:])
                std = smallp.tile([128, DT], F32, tag=f"sd{bi}")
                nc.scalar.activation(out=std[:], in_=gvar[:], func=AF.Sqrt,
                                     bias=epsbn[:], scale=1.0)
                rstd = smallp.tile([128, DT], F32, tag=f"rs{bi}")
                nc.vector.reciprocal(rstd[:], std[:])
                sc1 = smallp.tile([128, DT], F32, tag=f"s1{bi}")
                nc.vector.tensor_mul(sc1[:], bng_s[:], rstd[:])
                sh1 = smallp.tile([128, DT], F32, tag=f"h1{bi}")
                nc.vector.tensor_mul(sh1[:], gm[:], sc1[:])
                nc.vector.tensor_sub(sh1[:], bnb_s[:], sh1[:])
                sc2 = smallp.tile([128, DT], F32, tag=f"s2{bi}")
                nc.vector.tensor_scalar(out=sc2[:], in0=sc1[:], scalar1=alpha_s[:],
                                        scalar2=-1.0, op0=ALU.mult, op1=ALU.mult)
                sh2 = smallp.tile([128, DT], F32, tag=f"h2{bi}")
                nc.vector.tensor_scalar(out=sh2[:], in0=sh1[:], scalar1=alpha_s[:],
                                        scalar2=-1.0, op0=ALU.mult, op1=ALU.mult)
                return sc1, sh1, sc2, sh2

            # ---------- BN + PReLU apply (+x^T fold for branch a) ----------
            acc_lb = [None] * NLB

            def apply_lb(bi, coefs, lc):
                sc1, sh1, sc2, sh2 = coefs
                lsl = slice(lc * LBS, (lc + 1) * LBS)
                if bi == 0:
                    acc_lb[lc] = natap.tile([128, DT, LBS], BF16, tag="nata",
                                            name=f"acc{lc}")
                acc = acc_lb[lc]
                ybk = ybkp.tile([128, DT, LBS], BF16, tag="ybk")
                nc.gpsimd.dma_start(out=ybk[:], in_=yD[bi][:, :, lsl])
                for et in range(DT):
                    r1 = rtp.tile([128, LBS], BF16, tag="rt")
                    nc.scalar.activation(out=r1[:], in_=ybk[:, et, :], func=AF.Relu,
                                         scale=sc1[:, et:et + 1], bias=sh1[:, et:et + 1])
                    r2 = rtp.tile([128, LBS], BF16, tag="rt")
                    nc.scalar.activation(out=r2[:], in_=ybk[:, et, :], func=AF.Relu,
                                         scale=sc2[:, et:et + 1], bias=sh2[:, et:et + 1])
                    if bi == 0:
                        nc.vector.tensor_sub(acc[:, et, :], r1[:], r2[:])
                        nc.vector.tensor_add(acc[:, et, :], acc[:, et, :],
                                             qT[lc][:, et, :])
                    else:
                        nc.vector.tensor_add(acc[:, et, :], acc[:, et, :], r1[:])
                        nc.vector.tensor_sub(acc[:, et, :], acc[:, et, :], r2[:])

            # ---------- LN tail for the 4 l-tiles of one l-block ----------
            def tail_lb(lc):
                for j in range(CH):
                    lt = lc * CH + j
                    tp = ps_acc.tile([128, D], BF16, tag="acc", name=f"tp{lt}")
                    for dt in range(DT):
                        nc.tensor.matmul(
                            tp[:, dt * 128:(dt + 1) * 128],
                            acc_lb[lc][:, dt, j * 128:(j + 1) * 128], ident[:],
                            is_transpose=True, start=(dt == 0), stop=(dt == DT - 1),
                            skip_group_check=True)
                    lns = lnsp.tile([128, 2, 6], F32, tag="lns")
                    for g2 in range(2):
                        nc.vector.bn_stats(out=lns[:, g2, :],
                                           in_=tp[:, g2 * 384:(g2 + 1) * 384])
                    mvl = lnsp.tile([128, 2], F32, tag="mvl")
                    nc.vector.bn_aggr(out=mvl[:], in_=lns[:])
                    stdl = lnsp.tile([128, 1], F32, tag="stdl")
                    nc.scalar.activation(out=stdl[:], in_=mvl[:, 1:2], func=AF.Sqrt,
                                         bias=epsln[:], scale=1.0)
                    rstdl = lnsp.tile([128, 1], F32, tag="rstdl")
                    nc.vector.reciprocal(rstdl[:], stdl[:])
                    nbl = lnsp.tile([128, 1], F32, tag="nbl")
                    nc.vector.tensor_scalar(out=nbl[:], in0=mvl[:, 0:1],
                                            scalar1=rstdl[:], scalar2=-1.0,
                                            op0=ALU.mult, op1=ALU.mult)
                    onat = onatp.tile([128, D], F32, tag="onat")
                    nc.scalar.activation(out=onat[:], in_=tp[:], func=AF.Identity,
                                         scale=rstdl[:], bias=nbl[:])
                    nc.gpsimd.tensor_mul(onat[:], onat[:], lng_s[:])
                    nc.vector.tensor_add(onat[:], onat[:], lnb_s[:])
                    nc.sync.dma_start(out=out_d.ap()[lt * 128:(lt + 1) * 128, :],
                                      in_=onat[:])

            branch_compute(0, WTa, ba_s, kvT_a, nata)
            branch_compute(1, WTv, bv_s, kvT_v, natv)
            coefs_a = branch_coefs(0, bnag_s, bnab_s, pa_s)
            for lc in range(NLB):
                apply_lb(0, coefs_a, lc)
            coefs_v = branch_coefs(1, bnvg_s, bnvb_s, pv_s)
            for lc in range(NLB):
                apply_lb(1, coefs_v, lc)
                tail_lb(lc)

    nc.compile()
    return nc


def _get_nc():
    global _CACHED_NC
    if _CACHED_NC is None:
        _CACHED_NC = _build_nc()
    return _CACHED_NC


def kernel(**inputs):
    nc = _get_nc()
    x_a = np.asarray(inputs["x_a"], np.float32)
    x_v = np.asarray(inputs["x_v"], np.float32)
    x = np.asarray(inputs["x"], np.float32)
    shared = {
        "Wa": np.ascontiguousarray(inputs["W_a"], np.float32),
        "Wv": np.ascontiguousarray(inputs["W_v"], np.float32),
        "ba": np.ascontiguousarray(inputs["b_a"], np.float32),
        "bv": np.ascontiguousarray(inputs["b_v"], np.float32),
        "bnag": np.ascontiguousarray(inputs["bn_a_g"], np.float32),
        "bnab": np.ascontiguousarray(inputs["bn_a_b"], np.float32),
        "bnvg": np.ascontiguousarray(inputs["bn_v_g"], np.float32),
        "bnvb": np.ascontiguousarray(inputs["bn_v_b"], np.float32),
        "pa": np.ascontiguousarray(inputs["prelu_a"], np.float32),
        "pv": np.ascontiguousarray(inputs["prelu_v"], np.float32),
        "lng": np.ascontiguousarray(inputs["ln_g"], np.float32),
        "lnb": np.ascontiguousarray(inputs["ln_b"], np.float32),
    }
    in_maps = []
    for b in range(N_CORES):
        m = dict(shared)
        m["xq"] = np.ascontiguousarray(x[:, b, :])
        m["xa"] = np.ascontiguousarray(x_a[:, b, :])
        m["xv"] = np.ascontiguousarray(x_v[:, b, :])
        in_maps.append(m)
    trace = bool(int(os.environ.get("COATT_TRACE", "0")))
    res = run_bass_kernel_spmd(nc, in_maps, core_ids=list(range(N_CORES)),
                               trace=trace)
    kernel.last_results = res
    out = np.stack([res.results[b]["out"] for b in range(N_CORES)], axis=1)
    return out.astype(np.float32)
